# revision 7
# baseline (speedup 1.0000x reference)
"""Single-head causal attention (B=8, S=2048, D=1024, H=128) on 8 trn2 cores.

Data-parallel over batch (1 element per core). Per core, all matmuls run
single-pass in f32r (1 cycle/row on the PE for >=256-wide outputs, exact
fp32 numerics in this stack):

  P1: Q^T/K^T/V^T projections from x^T [D,S] f32r. The softmax scale
      (sqrt(D)=32) is folded into Q during the PSUM->SBUF copy on ACT
      (scale=32), and the biases ride the same copies (bias=[H,1] AP,
      bq pre-scaled by 32 on the host). V is stored bf16.
  P2: per 128-row strip i: scores = Q_strip.T @ K over the causal extent
      (spans chosen >=256 wide so f32r stays at 1 cyc/row), causal mask
      added on the diagonal span via a bf16 transpose-matmul, row max ->
      negated directly into the exp bias (reduce_max(negate=True)), exp
      on ACT writes P bf16 with accum_out producing per-chunk row sums
      (summed on the host), P^T via PE transpose + DVE copyback (or DMA
      xbar), PV accumulated per 512-wide band with V^T-transposed tiles.
      Output is written as out^T [H,S]; the host divides by the row sums.
"""
import os
import sys

sys.path.insert(0, "/opt/trn_rl_repo")
import numpy as np
import ml_dtypes

import concourse.bass as bass
import concourse.mybir as mybir
import concourse.tile as tile
from concourse import bacc
from concourse.bass_utils import run_bass_kernel_spmd
from concourse.masks import make_identity

B, S, D, H = 8, 2048, 1024, 128
NK = D // 128          # 8 d-tiles
NS = S // 128          # 16 strips
CH = 512               # psum chunk width
NCH = S // CH

F32 = mybir.dt.float32
F32R = mybir.dt.float32r
BF16 = mybir.dt.bfloat16

_NC_CACHE = {}


def _env(name, default):
    return os.environ.get(name, default)


def _spans_for(L):
    """Non-overlapping spans covering [0, L], all >=256 wide when possible
    (f32r matmul runs 4 cyc/row below 256), diagonal span last."""
    if L <= CH:
        return [(0, L)]
    rem = L % CH
    spans = []
    if rem == 0:
        first = CH
    elif rem == 128:
        first = 384  # last span will be 256
    else:
        first = rem  # 256 or 384
    spans.append((0, first))
    c0 = first
    while c0 < L:
        spans.append((c0, min(c0 + CH, L)))
        c0 += CH
    return spans


def _build():
    CBP = _env("CBP", "0") == "1"      # alternate P^T copybacks DVE/Pool
    PTX = int(_env("PTX", "99"))       # strips with i >= PTX use DMA xbar for P^T
    VX = _env("VX", "0") == "1"        # V transpose via DMA xbar
    PTW = int(_env("PTW", "1024"))     # ptb rolling window width (s cols)
    SCB = int(_env("SCB", "6"))
    PBUF = int(_env("PBUF", "3"))
    AUXB = int(_env("AUXB", "2"))
    OTB = int(_env("OTB", "2"))
    BDEF = _env("BDEF", "0") == "1"    # defer band PV one strip-group
    CBS = _env("CBS", "0") == "1"      # alternate P^T copybacks DVE/ACT

    nc = bacc.Bacc()
    xT_d = nc.declare_dram_parameter("xT", [D, S], F32R, isOutput=False)
    W_d = [nc.declare_dram_parameter(f"W{n}", [D, H], F32R, isOutput=False) for n in "qkv"]
    b_d = [nc.declare_dram_parameter(f"b{n}", [H, 1], F32, isOutput=False) for n in "qkv"]
    out_d = nc.declare_dram_parameter("outT", [H, S], F32, isOutput=True)
    sums_d = nc.declare_dram_parameter("sums", [128, NS * 4], F32, isOutput=True)
    DBG = _env("DBG", "0") == "1"
    if DBG:
        qt_dbg = nc.declare_dram_parameter("qt_dbg", [128, S], F32, isOutput=True)
        kt_dbg = nc.declare_dram_parameter("kt_dbg", [128, S], F32, isOutput=True)
        vt_dbg = nc.declare_dram_parameter("vt_dbg", [128, S], F32, isOutput=True)
        p10_dbg = nc.declare_dram_parameter("p10_dbg", [128, S], F32, isOutput=True)
        nb10_dbg = nc.declare_dram_parameter("nb10_dbg", [128, 1], F32, isOutput=True)

    with tile.TileContext(nc) as tc:
        with (
            tc.tile_pool(name="cons", bufs=1) as cons,
            tc.tile_pool(name="qkv", bufs=1) as qkv,
            tc.tile_pool(name="pp", bufs=2) as pp,
            tc.tile_pool(name="outp", bufs=4) as outp,
            tc.tile_pool(name="stat", bufs=8) as stat,
        ):
            # ---- constants ----
            w_sb = [cons.tile([128, NK, H], F32R, name=f"w{p}", tag=f"w{p}") for p in range(3)]
            b_sb = [cons.tile([128, 1], F32, name=f"b{p}", tag=f"b{p}") for p in range(3)]
            identb = cons.tile([128, 128], BF16, tag="identb")
            make_identity(nc, identb)
            identf = cons.tile([128, 128], F32, tag="identf")
            make_identity(nc, identf)
            # maskT[t, s] = -1e30 where s < t; its PE transpose is the
            # additive causal mask for the diagonal score tile.
            maskT = cons.tile([128, 128], F32, tag="maskT")
            nc.gpsimd.memset(maskT, 0.0)
            nc.gpsimd.affine_select(
                out=maskT, in_=maskT, compare_op=mybir.AluOpType.is_ge,
                fill=-1e30, base=0, pattern=[[1, 128]], channel_multiplier=-1,
            )

            qt = qkv.tile([128, S], F32R, tag="qt")
            kt = qkv.tile([128, S], F32R, tag="kt")
            vt_bf = qkv.tile([128, S], BF16, tag="vt")
            v_sb = qkv.tile([128, NS, H], BF16, tag="v")
            sums_sb = qkv.tile([128, NS * 4], F32, tag="sums")
            nc.gpsimd.memset(sums_sb, 0.0)

            with (
                tc.tile_pool(name="xtp", bufs=1) as xtp,
                tc.tile_pool(name="ps_a", bufs=SCB, space="PSUM") as ps_a,
            ):
                xt = [xtp.tile([128, S], F32R, name=f"xt{k}", tag=f"xt{k}") for k in range(NK)]
                HD = S // 2
                # ---- input DMA stream; first-needed first ----
                for p in range(3):
                    nc.sync.dma_start(out=w_sb[p][:, 0, :], in_=W_d[p][0:128, :])
                nc.sync.dma_start(out=xt[0][:, 0:CH], in_=xT_d[0:128, 0:CH])
                nc.sync.dma_start(out=xt[0][:, CH:HD], in_=xT_d[0:128, CH:HD])
                nc.sync.dma_start(out=xt[1][:, 0:HD], in_=xT_d[128:256, 0:HD])
                for p in range(3):
                    nc.sync.dma_start(
                        out=w_sb[p][:, 1:4, :],
                        in_=W_d[p][128:512, :].rearrange("(k p) h -> p k h", p=128),
                    )
                nc.sync.dma_start(out=xt[2][:, 0:HD], in_=xT_d[256:384, 0:HD])
                for p in range(3):
                    nc.sync.dma_start(
                        out=w_sb[p][:, 4:NK, :],
                        in_=W_d[p][512:1024, :].rearrange("(k p) h -> p k h", p=128),
                    )
                for k in range(3, NK):
                    nc.sync.dma_start(out=xt[k][:, 0:HD], in_=xT_d[128 * k : 128 * (k + 1), 0:HD])
                for p in range(3):
                    nc.sync.dma_start(out=b_sb[p], in_=b_d[p][:, :])
                for k in range(NK):
                    nc.sync.dma_start(out=xt[k][:, HD:S], in_=xT_d[128 * k : 128 * (k + 1), HD:S])

                ptb = pp.tile([128, NS, PTW], BF16, tag="pt", bufs=1)
                strip_p = {}

                def wcol(i):
                    return 128 * ((128 * i) % PTW // 128)

                def emit_proj(chunks):
                    psums = {}
                    for c in chunks:
                        for p in range(3):
                            psums[(c, p)] = ps_a.tile([128, CH], F32, name=f"pj{c}_{p}", tag="ps")
                    for k in range(NK):
                        for c in chunks:
                            for p in range(3):
                                nc.tensor.matmul(
                                    psums[(c, p)], w_sb[p][:, k, :],
                                    xt[k][:, CH * c : CH * (c + 1)],
                                    start=(k == 0), stop=(k == NK - 1),
                                )
                    for c in chunks:
                        sl = slice(CH * c, CH * (c + 1))
                        nc.scalar.activation(qt[:, sl], psums[(c, 0)],
                                             mybir.ActivationFunctionType.Identity,
                                             bias=b_sb[0], scale=32.0)
                        nc.scalar.activation(kt[:, sl], psums[(c, 1)],
                                             mybir.ActivationFunctionType.Identity,
                                             bias=b_sb[1], scale=1.0)
                        nc.scalar.activation(vt_bf[:, sl], psums[(c, 2)],
                                             mybir.ActivationFunctionType.Identity,
                                             bias=b_sb[2], scale=1.0)

                def emit_vtransp(j4):
                    vstage = ps_a.tile([128, 512], BF16, name=f"vst{j4}", tag="aux", bufs=AUXB)
                    for m in range(4):
                        j = j4 + m
                        nc.tensor.matmul(vstage[:, 128 * m : 128 * (m + 1)],
                                         vt_bf[:, 128 * j : 128 * (j + 1)], identb,
                                         is_transpose=True, start=True, stop=True,
                                         skip_group_check=True)
                    nc.vector.tensor_copy(v_sb[:, j4 : j4 + 4, :], vstage)

                def emit_vtransp_xbar():
                    nc.sync.dma_start(out=v_sb, in_=vt_bf, transpose=True)

                def emit_strip(i):
                    L = 128 * (i + 1)
                    qh = qt[:, 128 * i : 128 * (i + 1)]
                    spans = _spans_for(L)
                    scs = []
                    for (lo, hi) in spans:
                        w = hi - lo
                        sc = ps_a.tile([128, CH], F32, name=f"sc{i}_{lo}", tag="ps")
                        nc.tensor.matmul(sc[:, :w], qh, kt[:, lo:hi],
                                         start=True, stop=(hi != L))
                        if hi == L:
                            nc.tensor.matmul(sc[:, w - 128 : w], maskT, identf,
                                             is_transpose=True, start=False, stop=True,
                                             skip_group_check=True)
                        scs.append((sc, lo, w))
                    nch = len(scs)
                    nbias = stat.tile([128, 1], F32, name=f"nb{i}", tag=f"nb{i}")
                    if nch == 1:
                        nc.vector.reduce_max(out=nbias, in_=scs[0][0][:, : scs[0][2]],
                                             axis=mybir.AxisListType.X, negate=True)
                    else:
                        st = stat.tile([128, 4], F32, name=f"st{i}", tag=f"st{i}")
                        for c, (sc, lo, w) in enumerate(scs):
                            nc.vector.reduce_max(out=st[:, c : c + 1], in_=sc[:, :w],
                                                 axis=mybir.AxisListType.X)
                        nc.vector.reduce_max(out=nbias, in_=st[:, :nch],
                                             axis=mybir.AxisListType.X, negate=True)
                    p_sb = pp.tile([128, S], BF16, tag="p", bufs=PBUF)
                    strip_p[i] = p_sb
                    for c, (sc, lo, w) in enumerate(scs):
                        nc.scalar.activation(
                            p_sb[:, lo : lo + w], sc[:, :w],
                            mybir.ActivationFunctionType.Exp,
                            bias=nbias, scale=1.0,
                            accum_out=sums_sb[:, 4 * i + c : 4 * i + c + 1])
                    if DBG and i == 10:
                        pf = pp.tile([128, S], F32, tag="pdbg", bufs=1)
                        nc.vector.tensor_copy(pf[:, 0 : 128 * (i + 1)], p_sb[:, 0 : 128 * (i + 1)])
                        nc.gpsimd.memset(pf[:, 128 * (i + 1) : S], 0.0)
                        nc.sync.dma_start(out=p10_dbg[:, :], in_=pf)
                        nc.sync.dma_start(out=nb10_dbg[:, :], in_=nbias)

                def emit_strip_pt(i):
                    p_sb = strip_p[i]
                    wc = wcol(i)
                    if i >= PTX:
                        nc.sync.dma_start(
                            out=ptb[:, 0 : i + 1, wc : wc + 128],
                            in_=p_sb[:, 0 : 128 * (i + 1)], transpose=True)
                        return
                    for j4 in range(0, i + 1, 4):
                        jn = min(4, i + 1 - j4)
                        tstage = ps_a.tile([128, 512], BF16, name=f"tst{i}_{j4}",
                                           tag="aux", bufs=AUXB)
                        for m in range(jn):
                            j = j4 + m
                            nc.tensor.matmul(tstage[:, 128 * m : 128 * (m + 1)],
                                             p_sb[:, 128 * j : 128 * (j + 1)], identb,
                                             is_transpose=True, start=True, stop=True,
                                             skip_group_check=True)
                        dst = ptb[:, j4 : j4 + jn, wc : wc + 128]
                        srcv = tstage[:, : 128 * jn].rearrange("p (a b) -> p a b", b=128)
                        if CBS and (j4 // 4) % 2 == 1:
                            nc.scalar.activation(dst, srcv, mybir.ActivationFunctionType.Copy)
                        elif CBP and (j4 // 4) % 2 == 1:
                            nc.gpsimd.tensor_copy(dst, srcv)
                        else:
                            nc.vector.tensor_copy(dst, srcv)

                def emit_band(gi):
                    b_lo = 512 * gi
                    woff = b_lo % PTW
                    njs = 4 * gi + 4
                    oT = ps_a.tile([128, CH], F32, name=f"oT{gi}", tag="aux", bufs=OTB)
                    for j in range(njs):
                        lo = max(128 * j, b_lo) - b_lo
                        nc.tensor.matmul(oT[:, lo:], v_sb[:, j, :],
                                         ptb[:, j, woff + lo : woff + CH],
                                         start=(j == 0), stop=(j == njs - 1),
                                         skip_group_check=True)
                    osb = outp.tile([128, CH], F32, name=f"osb{gi}", tag="osb")
                    nc.vector.tensor_copy(osb, oT)
                    nc.sync.dma_start(out=out_d[:, b_lo : b_lo + CH], in_=osb)
                    if gi == 3:
                        nc.sync.dma_start(out=sums_d[:, :], in_=sums_sb)

                # ---- schedule ----
                emit_proj([0, 1])
                for i in (0, 1, 2, 3):
                    emit_strip(i)
                if not VX:
                    emit_vtransp(0)
                    emit_vtransp(4)
                emit_proj([2, 3])
                if VX:
                    emit_vtransp_xbar()
                else:
                    emit_vtransp(8)
                    emit_vtransp(12)
                for i in (0, 1, 2, 3):
                    emit_strip_pt(i)
                if not BDEF:
                    emit_band(0)
                for g in (1, 2, 3):
                    prev = None
                    for idx, i in enumerate(range(4 * g, 4 * g + 4)):
                        emit_strip(i)
                        if prev is not None:
                            emit_strip_pt(prev)
                        prev = i
                        if BDEF and idx == 1:
                            emit_band(g - 1)
                    emit_strip_pt(prev)
                    if not BDEF:
                        emit_band(g)
                if BDEF:
                    emit_band(3)
                if DBG:
                    qf = pp.tile([128, S], F32, tag="qdbg", bufs=1)
                    nc.vector.tensor_copy(qf, qt)
                    nc.sync.dma_start(out=qt_dbg[:, :], in_=qf)
                    nc.vector.tensor_copy(qf, kt)
                    nc.sync.dma_start(out=kt_dbg[:, :], in_=qf)
                    nc.vector.tensor_copy(qf, vt_bf)
                    nc.sync.dma_start(out=vt_dbg[:, :], in_=qf)

    nc.compile()
    return nc


def _get_nc():
    key = tuple(os.environ.get(k, d) for k, d in (
        ("CBP", "0"), ("PTX", "99"), ("VX", "0"), ("PTW", "1024"),
        ("SCB", "6"), ("PBUF", "3"), ("AUXB", "2"), ("OTB", "2"),
        ("BDEF", "0"), ("CBS", "0"), ("DBG", "0")))
    if key not in _NC_CACHE:
        _NC_CACHE[key] = _build()
    return _NC_CACHE[key]


def make_in_maps(x, Wq, bq, Wk, bk, Wv, bv):
    x = np.asarray(x, np.float32)
    xt = np.ascontiguousarray(x.transpose(0, 2, 1))  # [B, D, S]
    Ws = [np.ascontiguousarray(np.asarray(w, np.float32)) for w in (Wq, Wk, Wv)]
    bs = [np.ascontiguousarray(np.asarray(b, np.float32).reshape(H, 1)) for b in (bq, bk, bv)]
    bs[0] = np.ascontiguousarray(bs[0] * 32.0)
    in_maps = []
    for bi in range(B):
        m = {"xT": xt[bi]}
        for p, n in enumerate("qkv"):
            m[f"W{n}"] = Ws[p]
            m[f"b{n}"] = bs[p]
        in_maps.append(m)
    return in_maps


def kernel(x, Wq, bq, Wk, bk, Wv, bv):
    nc = _get_nc()
    in_maps = make_in_maps(x, Wq, bq, Wk, bk, Wv, bv)
    res = run_bass_kernel_spmd(nc, in_maps, list(range(B)))
    outs = []
    for b in range(B):
        oT = res.results[b]["outT"]                      # [H, S]
        sums = res.results[b]["sums"]                    # [128, NS*4]
        s = sums.reshape(128, NS, 4).sum(axis=-1)        # [128, NS]
        s_flat = s.T.reshape(S)                          # s = 128*i + p
        outs.append((oT / s_flat[None, :]).T)
    return np.stack(outs).astype(np.float32)
